# revision 4
# baseline (speedup 1.0000x reference)
"""Sparse Conv3d (3x3x3 kmap) + BatchNorm + ReLU on 8 TRN2 NeuronCores — v2.

Voxel/data parallel per the sharding hint: output voxels sharded 15000/core.
Off-center offsets use a per-core compacted bf16 source table (unique halo+
local sources, <32767 rows so indices fit int16 in ONE bank) and the
dma_gather transpose path: each gathered 256B token IS a matmul lhsT column
([64 cin on partitions 0:64, zeros 64:128]), so chunks of 128 tokens feed
  matmul(out=[128 tok, 64 cout], lhsT=gathered[:, chunk], rhs=Wk_stack)
directly — no PE transposes, no per-chunk fixup copies. Results are cast to
bf16 and dma_scatter_add'ed (parity-split SBUF CCE) into one of 4 accumulator
pairs; the 26 offsets are split into 4 groups on 4 SWDGE queues so the
scatter chains run concurrently. The center offset (identity map) is a plain
transposed matmul over a host-transposed bf16 slice, initializing pair 0.
BN stats come from ones/X^T X matmuls on the combined accumulator, an
AllReduce over the 8 cores, then an in-place affine+ReLU and bf16 output
(host upcasts to fp32).
"""

import sys
import os

for _p in ("/opt/trn_rl_repo", "/root/.axon_site/_ro/trn_rl_repo"):
    if os.path.isdir(_p) and _p not in sys.path:
        sys.path.insert(0, _p)

import numpy as np

N = 120000
CIN = 64
COUT = 64
K = 27
CENTER = 13
EPS = 1e-5
NCORES = 8
NC_ROWS = N // NCORES          # 15000
SLOTS = 118                    # ceil(15000/128); wrapped rows = 15104
WRAP_ROWS = SLOTS * 128        # 15104
TRASH = WRAP_ROWS - 1          # trash dst row (only ever receives zeros)
HGRP = (SLOTS + 1) // 2        # 59 groups per parity
NQ = 4                         # scatter groups == SWDGE queues


def _wrap16(idx):
    """Wrap an int stream into the [128, n/16] int16 layout dma_gather expects."""
    n = len(idx)
    assert n % 16 == 0
    w = np.ascontiguousarray(idx.reshape(n // 16, 16).T).astype(np.int16)
    return np.tile(w, (8, 1))


def _plan(nbr):
    """Host-side index preprocessing.

    Static (shared) metadata: per-offset chunk counts CK (max over cores),
    chunk->offset map, group split. Per-core: gather/scatter int16 streams and
    the local source row list for the compacted table."""
    offs = [k for k in range(K) if k != CENTER]
    pairs = {}                  # (c, k) -> (src_global, dst_local)
    cnt = np.zeros((NCORES, K), np.int64)
    for k in offs:
        v = nbr[k]
        for c in range(NCORES):
            seg = v[c * NC_ROWS:(c + 1) * NC_ROWS]
            val = np.nonzero(seg >= 0)[0]
            pairs[(c, k)] = (seg[val].astype(np.int64), val)
            cnt[c, k] = len(val)
    CK = {k: int(-(-cnt[:, k].max() // 128)) for k in offs}
    CK_tot = sum(CK.values())
    T_total = CK_tot * 128

    # split offsets into NQ groups balanced by chunk count
    order = sorted(offs, key=lambda k: -CK[k])
    groups = [[] for _ in range(NQ)]
    gload = [0] * NQ
    for k in order:
        g = int(np.argmin(gload))
        groups[g].append(k)
        gload[g] += CK[k]
    # keep original k order within groups (deterministic)
    groups = [sorted(g) for g in groups]
    # chunk layout: group-major, then k in group order
    k_seq = [k for g in groups for k in g]
    ck0 = {}
    p = 0
    for k in k_seq:
        ck0[k] = p
        p += CK[k]
    grp_tok0 = []
    grp_ntok = []
    p = 0
    for g in groups:
        grp_tok0.append(p * 128)
        ng = sum(CK[k] for k in g) * 128
        grp_ntok.append(ng)
        p += sum(CK[k] for k in g)

    # per-core local source tables + streams
    lt_rows = 0
    srcs_cores, gidx_cores, sidx_cores = [], [], []
    for c in range(NCORES):
        allsrc = np.concatenate([pairs[(c, k)][0] for k in offs])
        uniq = np.unique(allsrc)
        srcs_cores.append(uniq)
        lt_rows = max(lt_rows, len(uniq))
    LT = lt_rows + 1            # final row = zeros
    ZROW = LT - 1
    assert LT <= 32767, LT

    for c in range(NCORES):
        uniq = srcs_cores[c]
        gstream = np.full(T_total, ZROW, np.int64)
        sstream = np.full(T_total, TRASH, np.int64)
        for k in k_seq:
            src, dst = pairs[(c, k)]
            base = ck0[k] * 128
            loc = np.searchsorted(uniq, src)
            gstream[base:base + len(src)] = loc
            # dst (local row id) -> wrapped row id == same numbering (row r
            # of the core slice sits at wrapped position r)
            sstream[base:base + len(dst)] = dst
        gidx_cores.append(_wrap16(gstream))
        sidx_cores.append(_wrap16(sstream))

    meta = dict(offs=offs, CK=CK, CK_tot=CK_tot, T_total=T_total, LT=LT,
                groups=groups, k_seq=k_seq, ck0=ck0,
                grp_tok0=grp_tok0, grp_ntok=grp_ntok)
    return meta, gidx_cores, sidx_cores, srcs_cores


def _build_bass(meta):
    from concourse import mybir, bacc
    import concourse.tile as tile
    from concourse.masks import make_identity

    CK = meta["CK"]
    CK_tot = meta["CK_tot"]
    T_total = meta["T_total"]
    LT = meta["LT"]
    groups = meta["groups"]
    k_seq = meta["k_seq"]
    ck0 = meta["ck0"]
    grp_tok0 = meta["grp_tok0"]
    grp_ntok = meta["grp_ntok"]
    f32 = mybir.dt.float32
    bf16 = mybir.dt.bfloat16
    i16 = mybir.dt.int16
    offs = meta["offs"]

    nc = bacc.Bacc("TRN2", target_bir_lowering=False, debug=False,
                   num_devices=NCORES, num_swdge_queues=1)
    lt = nc.dram_tensor("lt", [LT, 128], bf16, kind="ExternalInput").ap()
    ftc = nc.dram_tensor("ftc", [CIN, WRAP_ROWS], bf16,
                         kind="ExternalInput").ap()
    wst = nc.dram_tensor("wst", [128, len(offs) * COUT], bf16,
                         kind="ExternalInput").ap()
    wc = nc.dram_tensor("wc", [CIN, COUT], bf16, kind="ExternalInput").ap()
    gidx = nc.dram_tensor("gidx", [128, T_total // 16], i16,
                          kind="ExternalInput").ap()
    sixd = nc.dram_tensor("sixd", [128, T_total // 16], i16,
                          kind="ExternalInput").ap()
    gbeta = nc.dram_tensor("gbeta", [1, 128], f32, kind="ExternalInput").ap()
    oute = nc.dram_tensor("oute", [128, HGRP, COUT], bf16,
                          kind="ExternalOutput").ap()
    outo = nc.dram_tensor("outo", [128, HGRP, COUT], bf16,
                          kind="ExternalOutput").ap()

    # offset -> column in wst
    kcol = {k: i for i, k in enumerate(offs)}

    with tile.TileContext(nc) as tc:
        with tc.tile_pool(name="sb", bufs=1) as pool, \
             tc.tile_pool(name="ps", bufs=2, space="PSUM") as psum, \
             tc.tile_pool(name="dram", bufs=1, space="DRAM") as dram:
            ident = pool.tile([128, 128], f32)
            make_identity(nc, ident[:])
            ones_b = pool.tile([128, 1], bf16)
            nc.vector.memset(ones_b[:], 1.0)
            onesr = pool.tile([1, 128], f32)
            nc.vector.memset(onesr[:], 1.0)
            istack = pool.tile([128, COUT], f32)
            nc.vector.tensor_copy(out=istack[0:64, :], in_=ident[0:64, 0:64])
            nc.vector.tensor_copy(out=istack[64:128, :],
                                  in_=ident[64:128, 64:128])

            gix = pool.tile([128, T_total // 16], i16)
            nc.sync.dma_start(out=gix[:], in_=gidx[:])
            six = pool.tile([128, T_total // 16], i16)
            nc.sync.dma_start(out=six[:], in_=sixd[:])
            wsb = pool.tile([128, len(offs) * COUT], bf16)
            nc.sync.dma_start(out=wsb[:], in_=wst[:])
            wcb = pool.tile([CIN, COUT], bf16)
            nc.sync.dma_start(out=wcb[:], in_=wc[:])
            gb = pool.tile([1, 128], f32)
            nc.sync.dma_start(out=gb[:], in_=gbeta[:])
            fts = pool.tile([CIN, WRAP_ROWS], bf16)
            nc.sync.dma_start(out=fts[:], in_=ftc[:])

            # 4 accumulator pairs (bf16). Pair 0 is initialized by the center
            # pass; pairs 1..3 are zeroed.
            aes = [pool.tile([128, HGRP, COUT], bf16, tag=f"ae{g}",
                             name=f"ae{g}") for g in range(NQ)]
            aos = [pool.tile([128, HGRP, COUT], bf16, tag=f"ao{g}",
                             name=f"ao{g}") for g in range(NQ)]
            for g in range(1, NQ):
                nc.scalar.memzero(aes[g][:])
                nc.scalar.memzero(aos[g][:])

            # ---- gathers: one per group, on its own SWDGE queue ----
            gths = []
            for g in range(NQ):
                gt = pool.tile([128, 1, grp_ntok[g]], bf16, tag=f"g{g}",
                               name=f"gth{g}")
                gths.append(gt)
                nc.gpsimd.dma_gather(
                    out_ap=gt[:], in_ap=lt[:],
                    idxs_ap=gix[:, grp_tok0[g] // 16:
                                (grp_tok0[g] + grp_ntok[g]) // 16],
                    num_idxs=grp_ntok[g], num_idxs_reg=grp_ntok[g],
                    elem_size=128, transpose=True, single_packet=False)

            # ---- center pass: matmul ftc columns with wc, init ae0/ao0 ----
            for j0 in range(0, SLOTS, 8):
                jn = min(8, SLOTS - j0)
                ne = (jn + 1) // 2
                no = jn // 2
                pc = psum.tile([128, 8, COUT], f32, tag="pc")
                for j in range(j0, j0 + jn):
                    lhsT = fts[:, j * 128:(j + 1) * 128]
                    if j % 2 == 0:
                        out_ap = pc[:, (j - j0) // 2, :]
                    else:
                        out_ap = pc[:, 4 + (j - j0) // 2, :]
                    nc.tensor.matmul(out=out_ap, lhsT=lhsT, rhs=wcb[:],
                                     start=True, stop=True)
                g0 = j0 // 2
                eng = nc.vector if (j0 // 8) % 2 == 0 else nc.scalar
                if eng is nc.vector:
                    nc.vector.tensor_copy(out=aes[0][:, g0:g0 + ne, :],
                                          in_=pc[:, 0:ne, :])
                    if no:
                        nc.vector.tensor_copy(out=aos[0][:, g0:g0 + no, :],
                                              in_=pc[:, 4:4 + no, :])
                else:
                    nc.scalar.copy(out=aes[0][:, g0:g0 + ne, :],
                                   in_=pc[:, 0:ne, :])
                    if no:
                        nc.scalar.copy(out=aos[0][:, g0:g0 + no, :],
                                       in_=pc[:, 4:4 + no, :])

            # ---- off-center: chunk matmuls -> Y (bf16) -> scatter-add ----
            ybuf = pool.tile([128, CK_tot, COUT], bf16)
            for g in range(NQ):
                gt = gths[g]
                base_ck = ck0[groups[g][0]]
                chunks = []     # (global chunk id, k)
                for k in groups[g]:
                    for j in range(CK[k]):
                        chunks.append((ck0[k] + j, k))
                for i0 in range(0, len(chunks), 8):
                    inb = min(8, len(chunks) - i0)
                    py = psum.tile([128, 8, COUT], f32, tag="py", bufs=4)
                    for q in range(inb):
                        cid, k = chunks[i0 + q]
                        loc = (cid - base_ck) * 128
                        nc.tensor.matmul(
                            out=py[:, q, :],
                            lhsT=gt[:, 0, loc:loc + 128],
                            rhs=wsb[:, kcol[k] * COUT:(kcol[k] + 1) * COUT],
                            start=True, stop=True)
                    c0 = chunks[i0][0]
                    if (i0 // 8) % 2 == 0:
                        nc.vector.tensor_copy(out=ybuf[:, c0:c0 + inb, :],
                                              in_=py[:, 0:inb, :])
                    else:
                        nc.scalar.copy(out=ybuf[:, c0:c0 + inb, :],
                                       in_=py[:, 0:inb, :])
                for k in groups[g]:
                    nc.gpsimd.dma_scatter_add(
                        out_ap=aes[g][:], in_ap=ybuf[:, ck0[k]:ck0[k] + CK[k], :],
                        idxs_ap=six[:, ck0[k] * 8:(ck0[k] + CK[k]) * 8],
                        num_idxs=CK[k] * 128, num_idxs_reg=CK[k] * 128,
                        elem_size=COUT, sbuf_tokens_per_rank=128,
                        parity_reg=0, out_ap_other=aos[g][:],
                        single_packet=False)

            # ---- combine pairs into pair 0 ----
            nc.vector.tensor_add(out=aes[1][:], in0=aes[1][:], in1=aes[2][:])
            nc.vector.tensor_add(out=aos[1][:], in0=aos[1][:], in1=aos[2][:])
            nc.vector.tensor_add(out=aes[0][:], in0=aes[0][:], in1=aes[3][:])
            nc.vector.tensor_add(out=aos[0][:], in0=aos[0][:], in1=aos[3][:])
            nc.vector.tensor_add(out=aes[0][:], in0=aes[0][:], in1=aes[1][:])
            nc.vector.tensor_add(out=aos[0][:], in0=aos[0][:], in1=aos[1][:])
            ae, ao = aes[0], aos[0]

            # ---- stats: sums + sum-squares over all rows ----
            # order: full-width [128,2,64] slices first and last so every
            # psum element's first write has start semantics and last write
            # carries stop (the [128,1,64] leftovers sit in the middle)
            pcov = psum.tile([128, 128], f32, tag="py", bufs=4)
            cov_ins = []
            for g0 in range(0, HGRP - 1, 2):
                cov_ins.append(ae[:, g0:g0 + 2, :])
            cov_ins.append(ae[:, HGRP - 1:HGRP, :])
            cov_ins.append(ao[:, HGRP - 1:HGRP, :])
            for g0 in range(0, HGRP - 1, 2):
                cov_ins.append(ao[:, g0:g0 + 2, :])
            for i, ap in enumerate(cov_ins):
                w = ap.shape[1] * COUT
                nc.tensor.matmul(out=pcov[0:w, 0:w], lhsT=ap, rhs=ap,
                                 start=(i == 0), stop=(i == len(cov_ins) - 1))
            psumr = psum.tile([1, 512], f32, tag="pc")
            sum_ins = []
            for g0 in range(0, HGRP - 8, 8):
                sum_ins.append(ae[:, g0:g0 + 8, :])
            sum_ins.append(ae[:, HGRP - (HGRP % 8 or 8):HGRP, :])
            sum_ins.append(ao[:, HGRP - (HGRP % 8 or 8):HGRP, :])
            for g0 in range(0, HGRP - 8, 8):
                sum_ins.append(ao[:, g0:g0 + 8, :])
            for i, ap in enumerate(sum_ins):
                w = ap.shape[1] * COUT
                nc.tensor.matmul(out=psumr[:, 0:w], lhsT=ones_b[:], rhs=ap,
                                 start=(i == 0), stop=(i == len(sum_ins) - 1))
            tmpc = pool.tile([128, 128], f32)
            nc.vector.tensor_mul(out=tmpc[:], in0=pcov[:], in1=ident[:])
            diagc = pool.tile([128, 1], f32)
            nc.vector.tensor_reduce(out=diagc[:], in_=tmpc[:],
                                    axis=mybir.AxisListType.X,
                                    op=mybir.AluOpType.add)
            psq = psum.tile([1, COUT], f32, tag="pq")
            nc.tensor.matmul(out=psq[:], lhsT=diagc[:], rhs=istack[:],
                             start=True, stop=True)
            ssum = pool.tile([1, 512], f32)
            nc.vector.tensor_copy(out=ssum[:], in_=psumr[:])
            nc.vector.tensor_add(out=ssum[:, 0:256], in0=ssum[:, 0:256],
                                 in1=ssum[:, 256:512])
            nc.vector.tensor_add(out=ssum[:, 0:128], in0=ssum[:, 0:128],
                                 in1=ssum[:, 128:256])
            nc.vector.tensor_add(out=ssum[:, 0:64], in0=ssum[:, 0:64],
                                 in1=ssum[:, 64:128])
            stats = pool.tile([1, 128], f32)
            nc.vector.tensor_copy(out=stats[:, 0:64], in_=ssum[:, 0:64])
            nc.vector.tensor_copy(out=stats[:, 64:128], in_=psq[:])

            # ---- AllReduce over 8 cores ----
            cin_d = dram.tile([1, 128], f32)
            cout_d = dram.tile([1, 128], f32, addr_space="Shared")
            nc.sync.dma_start(out=cin_d[:], in_=stats[:])
            if os.environ.get("BASS_SIM_NO_COLLECTIVE"):
                nc.sync.dma_start(out=cout_d[:], in_=cin_d[:])
            else:
                nc.gpsimd.collective_compute(
                    "AllReduce", mybir.AluOpType.add,
                    replica_groups=[list(range(NCORES))],
                    ins=[cin_d.opt()], outs=[cout_d.opt()])
            red = pool.tile([1, 128], f32)
            nc.sync.dma_start(out=red[:], in_=cout_d[:])

            # ---- affine params ----
            nscale = 1.0 / N
            if os.environ.get("BASS_SIM_NO_COLLECTIVE"):
                nscale = 1.0 / NC_ROWS
            mean = pool.tile([1, COUT], f32)
            nc.vector.tensor_scalar_mul(out=mean[:], in0=red[:, 0:64],
                                        scalar1=nscale)
            ex2 = pool.tile([1, COUT], f32)
            nc.vector.tensor_scalar_mul(out=ex2[:], in0=red[:, 64:128],
                                        scalar1=nscale)
            var = pool.tile([1, COUT], f32)
            nc.vector.tensor_mul(out=var[:], in0=mean[:], in1=mean[:])
            nc.vector.tensor_sub(out=var[:], in0=ex2[:], in1=var[:])
            nc.vector.tensor_scalar_add(out=var[:], in0=var[:], scalar1=EPS)
            std = pool.tile([1, COUT], f32)
            nc.scalar.sqrt(out=std[:], in_=var[:])
            rstd = pool.tile([1, COUT], f32)
            nc.vector.reciprocal(out=rstd[:], in_=std[:])
            scl = pool.tile([1, COUT], f32)
            nc.vector.tensor_mul(out=scl[:], in0=gb[:, 0:64], in1=rstd[:])
            bia = pool.tile([1, COUT], f32)
            nc.vector.tensor_mul(out=bia[:], in0=mean[:], in1=scl[:])
            nc.vector.tensor_sub(out=bia[:], in0=gb[:, 64:128], in1=bia[:])

            # broadcast to [128, 8, 64] bf16
            pbs = psum.tile([128, COUT], f32, tag="pq")
            nc.tensor.matmul(out=pbs[:], lhsT=onesr[:], rhs=scl[:],
                             start=True, stop=True)
            s8 = pool.tile([128, 8, COUT], bf16)
            nc.vector.tensor_copy(out=s8[:, 0, :], in_=pbs[:])
            pbb = psum.tile([128, COUT], f32, tag="pq")
            nc.tensor.matmul(out=pbb[:], lhsT=onesr[:], rhs=bia[:],
                             start=True, stop=True)
            b8 = pool.tile([128, 8, COUT], bf16)
            nc.vector.tensor_copy(out=b8[:, 0, :], in_=pbb[:])
            for t8 in (s8, b8):
                nc.vector.tensor_copy(out=t8[:, 1:2, :], in_=t8[:, 0:1, :])
                nc.vector.tensor_copy(out=t8[:, 2:4, :], in_=t8[:, 0:2, :])
                nc.vector.tensor_copy(out=t8[:, 4:8, :], in_=t8[:, 0:4, :])

            # ---- normalize + relu in place, then write out ----
            for t in (ae, ao):
                for g0 in range(0, HGRP, 8):
                    gn = min(8, HGRP - g0)
                    sl = t[:, g0:g0 + gn, :]
                    nc.vector.tensor_mul(out=sl, in0=sl, in1=s8[:, 0:gn, :])
                    nc.vector.tensor_add(out=sl, in0=sl, in1=b8[:, 0:gn, :])
                    nc.vector.tensor_scalar_max(out=sl, in0=sl, scalar1=0.0)
            nc.sync.dma_start(out=oute[:], in_=ae[:, :, :])
            nc.sync.dma_start(out=outo[:], in_=ao[:, :, :])

    nc.compile()
    return nc


def _host_tensors(feats, weight, gamma, beta, meta, srcs_cores):
    import ml_dtypes
    bf = ml_dtypes.bfloat16
    feats = np.ascontiguousarray(np.asarray(feats, dtype=np.float32))
    f16 = feats.astype(bf)
    weight = np.asarray(weight, dtype=np.float32)
    offs = meta["offs"]
    LT = meta["LT"]

    wstack = np.zeros((128, len(offs) * COUT), np.float32)
    for i, k in enumerate(offs):
        wstack[0:CIN, i * COUT:(i + 1) * COUT] = weight[k]
    wstack = wstack.astype(bf)
    wcv = weight[CENTER].astype(bf)

    gbv = np.zeros((1, 128), np.float32)
    gbv[0, 0:64] = np.asarray(gamma, np.float32)
    gbv[0, 64:128] = np.asarray(beta, np.float32)

    lts, ftcs = [], []
    for c in range(NCORES):
        t = np.zeros((LT, 128), bf)
        u = srcs_cores[c]
        t[:len(u), 0:CIN] = f16[u]
        lts.append(t)
        ft = np.zeros((CIN, WRAP_ROWS), bf)
        ft[:, :NC_ROWS] = f16[c * NC_ROWS:(c + 1) * NC_ROWS].T
        ftcs.append(ft)
    return lts, ftcs, wstack, wcv, gbv


def _prepare(np_inputs):
    nbr = np.asarray(np_inputs["neighbor_idx"])
    meta, gidx_cores, sidx_cores, srcs_cores = _plan(nbr)
    nc = _build_bass(meta)
    lts, ftcs, wstack, wcv, gbv = _host_tensors(
        np_inputs["feats"], np_inputs["weight"], np_inputs["gamma"],
        np_inputs["beta"], meta, srcs_cores)
    in_maps = [
        {"lt": lts[c], "ftc": ftcs[c], "wst": wstack, "wc": wcv,
         "gidx": gidx_cores[c], "sixd": sidx_cores[c], "gbeta": gbv}
        for c in range(NCORES)
    ]
    return nc, in_maps


def kernel(feats, weight, gamma, beta, neighbor_idx):
    from concourse.bass_utils import run_bass_kernel_spmd

    np_inputs = {"feats": feats, "weight": weight, "gamma": gamma,
                 "beta": beta, "neighbor_idx": neighbor_idx}
    nc, in_maps = _prepare(np_inputs)
    res = run_bass_kernel_spmd(nc, in_maps, core_ids=list(range(NCORES)))
    out = np.empty((N, COUT), np.float32)
    for c in range(NCORES):
        wrapped = np.empty((128, SLOTS, COUT), np.float32)
        wrapped[:, 0::2, :] = res.results[c]["oute"].astype(np.float32)
        wrapped[:, 1::2, :] = res.results[c]["outo"].astype(np.float32)
        rows = wrapped.transpose(1, 0, 2).reshape(WRAP_ROWS, COUT)
        out[c * NC_ROWS:(c + 1) * NC_ROWS] = rows[:NC_ROWS]
    return out


# revision 5
# speedup vs baseline: 1.5781x; 1.5781x over previous
"""Sparse Conv3d (3x3x3 kmap) + BatchNorm + ReLU on 8 TRN2 NeuronCores — v2.

Voxel/data parallel per the sharding hint: output voxels sharded 15000/core.
Off-center offsets use a per-core compacted bf16 source table (unique halo+
local sources, <32767 rows so indices fit int16 in ONE bank) and the
dma_gather transpose path: each gathered 256B token IS a matmul lhsT column
([64 cin on partitions 0:64, zeros 64:128]), so chunks of 128 tokens feed
  matmul(out=[128 tok, 64 cout], lhsT=gathered[:, chunk], rhs=Wk_stack)
directly — no PE transposes, no per-chunk fixup copies. Results are cast to
bf16 and dma_scatter_add'ed (parity-split SBUF CCE) into one of 4 accumulator
pairs; the 26 offsets are split into 4 groups on 4 SWDGE queues so the
scatter chains run concurrently. The center offset (identity map) is a plain
transposed matmul over a host-transposed bf16 slice, initializing pair 0.
BN stats come from ones/X^T X matmuls on the combined accumulator, an
AllReduce over the 8 cores, then an in-place affine+ReLU and bf16 output
(host upcasts to fp32).
"""

import sys
import os

for _p in ("/opt/trn_rl_repo", "/root/.axon_site/_ro/trn_rl_repo"):
    if os.path.isdir(_p) and _p not in sys.path:
        sys.path.insert(0, _p)

import numpy as np

N = 120000
CIN = 64
COUT = 64
K = 27
CENTER = 13
EPS = 1e-5
NCORES = 8
NC_ROWS = N // NCORES          # 15000
SLOTS = 118                    # ceil(15000/128); wrapped rows = 15104
WRAP_ROWS = SLOTS * 128        # 15104
TRASH = WRAP_ROWS - 1          # trash dst row (only ever receives zeros)
HGRP = (SLOTS + 1) // 2        # 59 groups per parity
NQ = 4                         # scatter groups == SWDGE queues


def _wrap16(idx):
    """Wrap an int stream into the [128, n/16] int16 layout dma_gather expects."""
    n = len(idx)
    assert n % 16 == 0
    w = np.ascontiguousarray(idx.reshape(n // 16, 16).T).astype(np.int16)
    return np.tile(w, (8, 1))


def _plan(nbr):
    """Host-side index preprocessing.

    Static (shared) metadata: per-offset chunk counts CK (max over cores),
    chunk->offset map, group split. Per-core: gather/scatter int16 streams and
    the local source row list for the compacted table."""
    offs = [k for k in range(K) if k != CENTER]
    pairs = {}                  # (c, k) -> (src_global, dst_local)
    cnt = np.zeros((NCORES, K), np.int64)
    for k in offs:
        v = nbr[k]
        for c in range(NCORES):
            seg = v[c * NC_ROWS:(c + 1) * NC_ROWS]
            val = np.nonzero(seg >= 0)[0]
            pairs[(c, k)] = (seg[val].astype(np.int64), val)
            cnt[c, k] = len(val)
    CK = {k: int(-(-cnt[:, k].max() // 128)) for k in offs}
    CK_tot = sum(CK.values())
    T_total = CK_tot * 128

    # split offsets into NQ groups balanced by chunk count
    order = sorted(offs, key=lambda k: -CK[k])
    groups = [[] for _ in range(NQ)]
    gload = [0] * NQ
    for k in order:
        g = int(np.argmin(gload))
        groups[g].append(k)
        gload[g] += CK[k]
    # keep original k order within groups (deterministic)
    groups = [sorted(g) for g in groups]
    # chunk layout: group-major, then k in group order
    k_seq = [k for g in groups for k in g]
    ck0 = {}
    p = 0
    for k in k_seq:
        ck0[k] = p
        p += CK[k]
    grp_tok0 = []
    grp_ntok = []
    p = 0
    for g in groups:
        grp_tok0.append(p * 128)
        ng = sum(CK[k] for k in g) * 128
        grp_ntok.append(ng)
        p += sum(CK[k] for k in g)

    # per-core local source tables + streams
    lt_rows = 0
    srcs_cores, gidx_cores, sidx_cores = [], [], []
    for c in range(NCORES):
        allsrc = np.concatenate([pairs[(c, k)][0] for k in offs])
        uniq = np.unique(allsrc)
        srcs_cores.append(uniq)
        lt_rows = max(lt_rows, len(uniq))
    LT = lt_rows + 1            # final row = zeros
    ZROW = LT - 1
    assert LT <= 32767, LT

    for c in range(NCORES):
        uniq = srcs_cores[c]
        gstream = np.full(T_total, ZROW, np.int64)
        sstream = np.full(T_total, TRASH, np.int64)
        for k in k_seq:
            src, dst = pairs[(c, k)]
            base = ck0[k] * 128
            loc = np.searchsorted(uniq, src)
            gstream[base:base + len(src)] = loc
            # dst (local row id) -> wrapped row id == same numbering (row r
            # of the core slice sits at wrapped position r)
            sstream[base:base + len(dst)] = dst
        gidx_cores.append(_wrap16(gstream))
        sidx_cores.append(_wrap16(sstream))

    meta = dict(offs=offs, CK=CK, CK_tot=CK_tot, T_total=T_total, LT=LT,
                groups=groups, k_seq=k_seq, ck0=ck0,
                grp_tok0=grp_tok0, grp_ntok=grp_ntok)
    return meta, gidx_cores, sidx_cores, srcs_cores


def _build_bass(meta):
    from concourse import mybir, bacc
    import concourse.tile as tile
    from concourse.masks import make_identity

    CK = meta["CK"]
    CK_tot = meta["CK_tot"]
    T_total = meta["T_total"]
    LT = meta["LT"]
    groups = meta["groups"]
    k_seq = meta["k_seq"]
    ck0 = meta["ck0"]
    grp_tok0 = meta["grp_tok0"]
    grp_ntok = meta["grp_ntok"]
    f32 = mybir.dt.float32
    bf16 = mybir.dt.bfloat16
    i16 = mybir.dt.int16
    offs = meta["offs"]

    nc = bacc.Bacc("TRN2", target_bir_lowering=False, debug=False,
                   num_devices=NCORES, num_swdge_queues=1)
    lt = nc.dram_tensor("lt", [LT, 128], bf16, kind="ExternalInput").ap()
    ftc = nc.dram_tensor("ftc", [CIN, WRAP_ROWS], bf16,
                         kind="ExternalInput").ap()
    wst = nc.dram_tensor("wst", [128, len(offs) * COUT], bf16,
                         kind="ExternalInput").ap()
    wc = nc.dram_tensor("wc", [CIN, COUT], bf16, kind="ExternalInput").ap()
    gidx = nc.dram_tensor("gidx", [128, T_total // 16], i16,
                          kind="ExternalInput").ap()
    sixd = nc.dram_tensor("sixd", [128, T_total // 16], i16,
                          kind="ExternalInput").ap()
    gbeta = nc.dram_tensor("gbeta", [1, 128], f32, kind="ExternalInput").ap()
    oute = nc.dram_tensor("oute", [128, HGRP, COUT], bf16,
                          kind="ExternalOutput").ap()
    outo = nc.dram_tensor("outo", [128, HGRP, COUT], bf16,
                          kind="ExternalOutput").ap()

    # offset -> column in wst
    kcol = {k: i for i, k in enumerate(offs)}

    with tile.TileContext(nc) as tc:
        with tc.tile_pool(name="sb", bufs=1) as pool, \
             tc.tile_pool(name="ps", bufs=2, space="PSUM") as psum, \
             tc.tile_pool(name="dram", bufs=1, space="DRAM") as dram:
            ident = pool.tile([128, 128], f32)
            make_identity(nc, ident[:])
            ones_b = pool.tile([128, 1], bf16)
            nc.vector.memset(ones_b[:], 1.0)
            onesr = pool.tile([1, 128], f32)
            nc.vector.memset(onesr[:], 1.0)
            istack = pool.tile([128, COUT], f32)
            nc.vector.tensor_copy(out=istack[0:64, :], in_=ident[0:64, 0:64])
            nc.vector.tensor_copy(out=istack[64:128, :],
                                  in_=ident[64:128, 64:128])

            gix = pool.tile([128, T_total // 16], i16)
            nc.sync.dma_start(out=gix[:], in_=gidx[:])
            six = pool.tile([128, T_total // 16], i16)
            nc.sync.dma_start(out=six[:], in_=sixd[:])
            wsb = pool.tile([128, len(offs) * COUT], bf16)
            nc.sync.dma_start(out=wsb[:], in_=wst[:])
            wcb = pool.tile([CIN, COUT], bf16)
            nc.sync.dma_start(out=wcb[:], in_=wc[:])
            gb = pool.tile([1, 128], f32)
            nc.sync.dma_start(out=gb[:], in_=gbeta[:])
            fts = pool.tile([CIN, WRAP_ROWS], bf16)
            nc.sync.dma_start(out=fts[:], in_=ftc[:])

            # 4 accumulator pairs (bf16). Pair 0 is initialized by the center
            # pass; pairs 1..3 are zeroed.
            aes = [pool.tile([128, HGRP, COUT], bf16, tag=f"ae{g}",
                             name=f"ae{g}") for g in range(NQ)]
            aos = [pool.tile([128, HGRP, COUT], bf16, tag=f"ao{g}",
                             name=f"ao{g}") for g in range(NQ)]
            for g in range(1, NQ):
                nc.scalar.memzero(aes[g][:])
                nc.scalar.memzero(aos[g][:])

            # ---- gathers: one per group, on its own SWDGE queue ----
            gths = []
            for g in range(NQ):
                gt = pool.tile([128, 1, grp_ntok[g]], bf16, tag=f"g{g}",
                               name=f"gth{g}")
                gths.append(gt)
                nc.gpsimd.dma_gather(
                    out_ap=gt[:], in_ap=lt[:],
                    idxs_ap=gix[:, grp_tok0[g] // 16:
                                (grp_tok0[g] + grp_ntok[g]) // 16],
                    num_idxs=grp_ntok[g], num_idxs_reg=grp_ntok[g],
                    elem_size=128, transpose=True, single_packet=False)

            # ---- center pass: matmul ftc columns with wc, init ae0/ao0 ----
            for j0 in range(0, SLOTS, 8):
                jn = min(8, SLOTS - j0)
                ne = (jn + 1) // 2
                no = jn // 2
                pc = psum.tile([128, 8, COUT], f32, tag="pc")
                for j in range(j0, j0 + jn):
                    lhsT = fts[:, j * 128:(j + 1) * 128]
                    if j % 2 == 0:
                        out_ap = pc[:, (j - j0) // 2, :]
                    else:
                        out_ap = pc[:, 4 + (j - j0) // 2, :]
                    nc.tensor.matmul(out=out_ap, lhsT=lhsT, rhs=wcb[:],
                                     start=True, stop=True)
                g0 = j0 // 2
                eng = nc.vector if (j0 // 8) % 2 == 0 else nc.scalar
                if eng is nc.vector:
                    nc.vector.tensor_copy(out=aes[0][:, g0:g0 + ne, :],
                                          in_=pc[:, 0:ne, :])
                    if no:
                        nc.vector.tensor_copy(out=aos[0][:, g0:g0 + no, :],
                                              in_=pc[:, 4:4 + no, :])
                else:
                    nc.scalar.copy(out=aes[0][:, g0:g0 + ne, :],
                                   in_=pc[:, 0:ne, :])
                    if no:
                        nc.scalar.copy(out=aos[0][:, g0:g0 + no, :],
                                       in_=pc[:, 4:4 + no, :])

            # ---- off-center: chunk matmuls -> Y (bf16) -> scatter-add ----
            ybuf = pool.tile([128, CK_tot, COUT], bf16)
            for g in range(NQ):
                gt = gths[g]
                base_ck = ck0[groups[g][0]]
                chunks = []     # (global chunk id, k)
                for k in groups[g]:
                    for j in range(CK[k]):
                        chunks.append((ck0[k] + j, k))
                for i0 in range(0, len(chunks), 8):
                    inb = min(8, len(chunks) - i0)
                    py = psum.tile([128, 8, COUT], f32, tag="py", bufs=4)
                    for q in range(inb):
                        cid, k = chunks[i0 + q]
                        loc = (cid - base_ck) * 128
                        nc.tensor.matmul(
                            out=py[:, q, :],
                            lhsT=gt[:, 0, loc:loc + 128],
                            rhs=wsb[:, kcol[k] * COUT:(kcol[k] + 1) * COUT],
                            start=True, stop=True)
                    c0 = chunks[i0][0]
                    if (i0 // 8) % 2 == 0:
                        nc.vector.tensor_copy(out=ybuf[:, c0:c0 + inb, :],
                                              in_=py[:, 0:inb, :])
                    else:
                        nc.scalar.copy(out=ybuf[:, c0:c0 + inb, :],
                                       in_=py[:, 0:inb, :])
                for k in groups[g]:
                    nc.gpsimd.dma_scatter_add(
                        out_ap=aes[g][:], in_ap=ybuf[:, ck0[k]:ck0[k] + CK[k], :],
                        idxs_ap=six[:, ck0[k] * 8:(ck0[k] + CK[k]) * 8],
                        num_idxs=CK[k] * 128, num_idxs_reg=CK[k] * 128,
                        elem_size=COUT, sbuf_tokens_per_rank=128,
                        parity_reg=0, out_ap_other=aos[g][:],
                        single_packet=False)

            # ---- combine pairs into pair 0 ----
            nc.vector.tensor_add(out=aes[1][:], in0=aes[1][:], in1=aes[2][:])
            nc.vector.tensor_add(out=aos[1][:], in0=aos[1][:], in1=aos[2][:])
            nc.vector.tensor_add(out=aes[0][:], in0=aes[0][:], in1=aes[3][:])
            nc.vector.tensor_add(out=aos[0][:], in0=aos[0][:], in1=aos[3][:])
            nc.vector.tensor_add(out=aes[0][:], in0=aes[0][:], in1=aes[1][:])
            nc.vector.tensor_add(out=aos[0][:], in0=aos[0][:], in1=aos[1][:])
            ae, ao = aes[0], aos[0]

            # ---- stats: sums + sum-squares over all rows ----
            # order: full-width [128,2,64] slices first and last so every
            # psum element's first write has start semantics and last write
            # carries stop (the [128,1,64] leftovers sit in the middle)
            pcov = psum.tile([128, 128], f32, tag="py", bufs=4)
            cov_ins = []
            for g0 in range(0, HGRP - 1, 2):
                cov_ins.append(ae[:, g0:g0 + 2, :])
            cov_ins.append(ae[:, HGRP - 1:HGRP, :])
            cov_ins.append(ao[:, HGRP - 1:HGRP, :])
            for g0 in range(0, HGRP - 1, 2):
                cov_ins.append(ao[:, g0:g0 + 2, :])
            for i, ap in enumerate(cov_ins):
                w = ap.shape[1] * COUT
                nc.tensor.matmul(out=pcov[0:w, 0:w], lhsT=ap, rhs=ap,
                                 start=(i == 0), stop=(i == len(cov_ins) - 1))
            psumr = psum.tile([1, 512], f32, tag="pc")
            sum_ins = []
            for g0 in range(0, HGRP - 8, 8):
                sum_ins.append(ae[:, g0:g0 + 8, :])
            sum_ins.append(ae[:, HGRP - (HGRP % 8 or 8):HGRP, :])
            sum_ins.append(ao[:, HGRP - (HGRP % 8 or 8):HGRP, :])
            for g0 in range(0, HGRP - 8, 8):
                sum_ins.append(ao[:, g0:g0 + 8, :])
            for i, ap in enumerate(sum_ins):
                w = ap.shape[1] * COUT
                nc.tensor.matmul(out=psumr[:, 0:w], lhsT=ones_b[:], rhs=ap,
                                 start=(i == 0), stop=(i == len(sum_ins) - 1))
            tmpc = pool.tile([128, 128], f32)
            nc.vector.tensor_mul(out=tmpc[:], in0=pcov[:], in1=ident[:])
            diagc = pool.tile([128, 1], f32)
            nc.vector.tensor_reduce(out=diagc[:], in_=tmpc[:],
                                    axis=mybir.AxisListType.X,
                                    op=mybir.AluOpType.add)
            psq = psum.tile([1, COUT], f32, tag="pq")
            nc.tensor.matmul(out=psq[:], lhsT=diagc[:], rhs=istack[:],
                             start=True, stop=True)
            ssum = pool.tile([1, 512], f32)
            nc.vector.tensor_copy(out=ssum[:], in_=psumr[:])
            nc.vector.tensor_add(out=ssum[:, 0:256], in0=ssum[:, 0:256],
                                 in1=ssum[:, 256:512])
            nc.vector.tensor_add(out=ssum[:, 0:128], in0=ssum[:, 0:128],
                                 in1=ssum[:, 128:256])
            nc.vector.tensor_add(out=ssum[:, 0:64], in0=ssum[:, 0:64],
                                 in1=ssum[:, 64:128])
            stats = pool.tile([1, 128], f32)
            nc.vector.tensor_copy(out=stats[:, 0:64], in_=ssum[:, 0:64])
            nc.vector.tensor_copy(out=stats[:, 64:128], in_=psq[:])

            # ---- AllReduce over 8 cores: recursive doubling via three
            # 2-rank collectives (strides 1, 2, 4) ----
            cin_d = dram.tile([1, 128], f32)
            c1_d = dram.tile([1, 128], f32)
            c2_d = dram.tile([1, 128], f32)
            cout_d = dram.tile([1, 128], f32)
            nc.sync.dma_start(out=cin_d[:], in_=stats[:])
            if os.environ.get("BASS_SIM_NO_COLLECTIVE"):
                nc.sync.dma_start(out=cout_d[:], in_=cin_d[:])
            else:
                nc.gpsimd.collective_compute(
                    "AllReduce", mybir.AluOpType.add,
                    replica_groups=[[0, 1], [2, 3], [4, 5], [6, 7]],
                    ins=[cin_d.opt()], outs=[c1_d.opt()])
                nc.gpsimd.collective_compute(
                    "AllReduce", mybir.AluOpType.add,
                    replica_groups=[[0, 2], [1, 3], [4, 6], [5, 7]],
                    ins=[c1_d.opt()], outs=[c2_d.opt()])
                nc.gpsimd.collective_compute(
                    "AllReduce", mybir.AluOpType.add,
                    replica_groups=[[0, 4], [1, 5], [2, 6], [3, 7]],
                    ins=[c2_d.opt()], outs=[cout_d.opt()])
            red = pool.tile([1, 128], f32)
            nc.sync.dma_start(out=red[:], in_=cout_d[:])

            # ---- affine params ----
            nscale = 1.0 / N
            if os.environ.get("BASS_SIM_NO_COLLECTIVE"):
                nscale = 1.0 / NC_ROWS
            mean = pool.tile([1, COUT], f32)
            nc.vector.tensor_scalar_mul(out=mean[:], in0=red[:, 0:64],
                                        scalar1=nscale)
            ex2 = pool.tile([1, COUT], f32)
            nc.vector.tensor_scalar_mul(out=ex2[:], in0=red[:, 64:128],
                                        scalar1=nscale)
            var = pool.tile([1, COUT], f32)
            nc.vector.tensor_mul(out=var[:], in0=mean[:], in1=mean[:])
            nc.vector.tensor_sub(out=var[:], in0=ex2[:], in1=var[:])
            nc.vector.tensor_scalar_add(out=var[:], in0=var[:], scalar1=EPS)
            std = pool.tile([1, COUT], f32)
            nc.scalar.sqrt(out=std[:], in_=var[:])
            rstd = pool.tile([1, COUT], f32)
            nc.vector.reciprocal(out=rstd[:], in_=std[:])
            scl = pool.tile([1, COUT], f32)
            nc.vector.tensor_mul(out=scl[:], in0=gb[:, 0:64], in1=rstd[:])
            bia = pool.tile([1, COUT], f32)
            nc.vector.tensor_mul(out=bia[:], in0=mean[:], in1=scl[:])
            nc.vector.tensor_sub(out=bia[:], in0=gb[:, 64:128], in1=bia[:])

            # broadcast to [128, 8, 64] bf16
            pbs = psum.tile([128, COUT], f32, tag="pq")
            nc.tensor.matmul(out=pbs[:], lhsT=onesr[:], rhs=scl[:],
                             start=True, stop=True)
            s8 = pool.tile([128, 8, COUT], bf16)
            nc.vector.tensor_copy(out=s8[:, 0, :], in_=pbs[:])
            pbb = psum.tile([128, COUT], f32, tag="pq")
            nc.tensor.matmul(out=pbb[:], lhsT=onesr[:], rhs=bia[:],
                             start=True, stop=True)
            b8 = pool.tile([128, 8, COUT], bf16)
            nc.vector.tensor_copy(out=b8[:, 0, :], in_=pbb[:])
            for t8 in (s8, b8):
                nc.vector.tensor_copy(out=t8[:, 1:2, :], in_=t8[:, 0:1, :])
                nc.vector.tensor_copy(out=t8[:, 2:4, :], in_=t8[:, 0:2, :])
                nc.vector.tensor_copy(out=t8[:, 4:8, :], in_=t8[:, 0:4, :])

            # ---- normalize + relu in place, then write out ----
            for t in (ae, ao):
                for g0 in range(0, HGRP, 8):
                    gn = min(8, HGRP - g0)
                    sl = t[:, g0:g0 + gn, :]
                    nc.vector.tensor_mul(out=sl, in0=sl, in1=s8[:, 0:gn, :])
                    nc.vector.tensor_add(out=sl, in0=sl, in1=b8[:, 0:gn, :])
                    nc.vector.tensor_scalar_max(out=sl, in0=sl, scalar1=0.0)
            nc.sync.dma_start(out=oute[:], in_=ae[:, :, :])
            nc.sync.dma_start(out=outo[:], in_=ao[:, :, :])

    nc.compile()
    return nc


def _host_tensors(feats, weight, gamma, beta, meta, srcs_cores):
    import ml_dtypes
    bf = ml_dtypes.bfloat16
    feats = np.ascontiguousarray(np.asarray(feats, dtype=np.float32))
    f16 = feats.astype(bf)
    weight = np.asarray(weight, dtype=np.float32)
    offs = meta["offs"]
    LT = meta["LT"]

    wstack = np.zeros((128, len(offs) * COUT), np.float32)
    for i, k in enumerate(offs):
        wstack[0:CIN, i * COUT:(i + 1) * COUT] = weight[k]
    wstack = wstack.astype(bf)
    wcv = weight[CENTER].astype(bf)

    gbv = np.zeros((1, 128), np.float32)
    gbv[0, 0:64] = np.asarray(gamma, np.float32)
    gbv[0, 64:128] = np.asarray(beta, np.float32)

    lts, ftcs = [], []
    for c in range(NCORES):
        t = np.zeros((LT, 128), bf)
        u = srcs_cores[c]
        t[:len(u), 0:CIN] = f16[u]
        lts.append(t)
        ft = np.zeros((CIN, WRAP_ROWS), bf)
        ft[:, :NC_ROWS] = f16[c * NC_ROWS:(c + 1) * NC_ROWS].T
        ftcs.append(ft)
    return lts, ftcs, wstack, wcv, gbv


def _prepare(np_inputs):
    nbr = np.asarray(np_inputs["neighbor_idx"])
    meta, gidx_cores, sidx_cores, srcs_cores = _plan(nbr)
    nc = _build_bass(meta)
    lts, ftcs, wstack, wcv, gbv = _host_tensors(
        np_inputs["feats"], np_inputs["weight"], np_inputs["gamma"],
        np_inputs["beta"], meta, srcs_cores)
    in_maps = [
        {"lt": lts[c], "ftc": ftcs[c], "wst": wstack, "wc": wcv,
         "gidx": gidx_cores[c], "sixd": sidx_cores[c], "gbeta": gbv}
        for c in range(NCORES)
    ]
    return nc, in_maps


def kernel(feats, weight, gamma, beta, neighbor_idx):
    from concourse.bass_utils import run_bass_kernel_spmd

    np_inputs = {"feats": feats, "weight": weight, "gamma": gamma,
                 "beta": beta, "neighbor_idx": neighbor_idx}
    nc, in_maps = _prepare(np_inputs)
    res = run_bass_kernel_spmd(nc, in_maps, core_ids=list(range(NCORES)))
    out = np.empty((N, COUT), np.float32)
    for c in range(NCORES):
        wrapped = np.empty((128, SLOTS, COUT), np.float32)
        wrapped[:, 0::2, :] = res.results[c]["oute"].astype(np.float32)
        wrapped[:, 1::2, :] = res.results[c]["outo"].astype(np.float32)
        rows = wrapped.transpose(1, 0, 2).reshape(WRAP_ROWS, COUT)
        out[c * NC_ROWS:(c + 1) * NC_ROWS] = rows[:NC_ROWS]
    return out
